# revision 1
# baseline (speedup 1.0000x reference)
"""Trainium2 Bass kernel: GroupNorm + cross-attention block (nn_CrossAttention).

Computation per batch b:
  xn   = GroupNorm32(x[b]) * gn_w + gn_b              # x: (512, 64*64)
  q    = wq @ xn + bq                                  # (512, 4096)
  cn   = LayerNorm(context[b]) * ln_w + ln_b           # (256, 768)
  k, v = split(cn @ wkv.T + bkv)                       # (256, 512) each
  sim  = q^T k^T * c^-0.5 ; attn = softmax_j(sim)      # (4096, 256)
  out  = wo @ (attn @ v)^T + bo + x[b]                 # (512, 4096)

Sharding: data-parallel over batch B=16 across 8 NeuronCores (2 batches/core).

Layout strategy (per core, per batch):
  - channels live on SBUF partitions (4 tiles of 128); hw positions stream on
    the free axis in 8 i-tiles of 512.
  - GroupNorm folded into the Q projection: wq' = wq * A[c], q0 = wq @ B where
    xn = x*A + B; stats come from a bn_stats pass + tiny cross-partition
    group-reduce matmuls against 0/1 selector matrices.
  - attention computed transposed (sim^T: context j on partitions, hw i on the
    free axis) so softmax's reductions over j become ones-matmuls on the PE and
    no transposes of the 4096x256 attention matrix are ever needed.
  - all large matmuls run in float32r (1 cycle/row on the PE vs 4 for fp32,
    ~1e-4 matmul rel err); stats/small matmuls stay fp32.
"""

import numpy as np

# problem shapes (hardcoded per contract)
B, C, HGT, WID = 16, 512, 64, 64
HW = HGT * WID            # 4096
S, CTX = 256, 768
G = 32                    # groups
GS = C // G               # 16 channels per group
EPS = 1e-5
NCORES = 8
BPC = B // NCORES         # batches per core = 2
P = 128
CT = C // P               # 4 channel tiles
KTC = CTX // P            # 6 ctx k-tiles
ST = S // P               # 2 seq tiles
ITW = 512                 # i-tile width (hw positions)
NIT = HW // ITW           # 8 i-tiles
SCALE = float(C) ** -0.5

_CACHE: dict = {}


def build_nc(reps: int = 1):
    """Build (and cache) the Bass module for one core's shard.

    reps>1 repeats the whole computation back-to-back inside one NEFF —
    used only by the timing harness to amortize dispatch overhead."""
    key = ("nc", reps)
    if key in _CACHE:
        return _CACHE[key]

    import concourse.bacc as bacc
    import concourse.mybir as mybir
    import concourse.tile as tile

    f32 = mybir.dt.float32
    f32r = mybir.dt.float32r
    AF = mybir.ActivationFunctionType
    OP = mybir.AluOpType

    nc = bacc.Bacc(None, target_bir_lowering=False)

    # ---- external I/O ----------------------------------------------------
    x_d = nc.declare_dram_parameter("x", [BPC, CT, P, HW], f32r, isOutput=False)
    ctx_d = nc.declare_dram_parameter("ctx", [BPC, ST, P, CTX], f32, isOutput=False)
    wqt_d = nc.declare_dram_parameter("wqt", [CT, P, C], f32, isOutput=False)
    wkvt_d = nc.declare_dram_parameter("wkvt", [KTC, P, 2 * C], f32r, isOutput=False)
    wot_d = nc.declare_dram_parameter("wot", [CT, P, C], f32r, isOutput=False)
    bq_d = nc.declare_dram_parameter("bq_p", [P, CT], f32, isOutput=False)
    bo_d = nc.declare_dram_parameter("bo_p", [P, CT], f32, isOutput=False)
    bk_d = nc.declare_dram_parameter("bk_p", [P, CT], f32, isOutput=False)
    gnw_d = nc.declare_dram_parameter("gnw_p", [P, CT], f32, isOutput=False)
    gnb_d = nc.declare_dram_parameter("gnb_p", [P, CT], f32, isOutput=False)
    bv_d = nc.declare_dram_parameter("bv_b", [P, C], f32, isOutput=False)
    lnw_d = nc.declare_dram_parameter("lnw_b", [P, CTX], f32, isOutput=False)
    lnb_d = nc.declare_dram_parameter("lnb_b", [P, CTX], f32, isOutput=False)
    sel_d = nc.declare_dram_parameter("sel", [P, G // CT], f32, isOutput=False)
    selt_d = nc.declare_dram_parameter("selt", [G // CT, P], f32, isOutput=False)
    id_d = nc.declare_dram_parameter("ident", [P, P], f32, isOutput=False)
    out_d = nc.declare_dram_parameter("out", [BPC, CT, P, HW], f32, isOutput=True)

    GT = G // CT  # 8 groups per channel tile

    with tile.TileContext(nc) as tc:
        with (
            tc.tile_pool(name="persist", bufs=1) as pp,
            tc.tile_pool(name="bpool", bufs=2) as bp,
            tc.tile_pool(name="xpool", bufs=3) as xp,
            tc.tile_pool(name="ipool", bufs=2) as ip,
            tc.tile_pool(name="ps", bufs=8, space="PSUM") as ps,
        ):
            # ---- persistent loads ----------------------------------------
            wqt_sb = pp.tile([P, CT, C], f32)
            nc.sync.dma_start(wqt_sb, wqt_d[:].rearrange("t p c -> p t c"))
            wkvt_sb = pp.tile([P, KTC, 2 * C], f32r)
            nc.sync.dma_start(wkvt_sb, wkvt_d[:].rearrange("t p c -> p t c"))
            wot_sb = pp.tile([P, CT, C], f32r)
            nc.sync.dma_start(wot_sb, wot_d[:].rearrange("t p c -> p t c"))
            bq_sb = pp.tile([P, CT], f32)
            nc.sync.dma_start(bq_sb, bq_d[:])
            bo_sb = pp.tile([P, CT], f32)
            nc.sync.dma_start(bo_sb, bo_d[:])
            bk_sb = pp.tile([P, CT], f32)
            nc.sync.dma_start(bk_sb, bk_d[:])
            gnw_sb = pp.tile([P, CT], f32)
            nc.sync.dma_start(gnw_sb, gnw_d[:])
            gnb_sb = pp.tile([P, CT], f32)
            nc.sync.dma_start(gnb_sb, gnb_d[:])
            bv_sb = pp.tile([P, C], f32)
            nc.sync.dma_start(bv_sb, bv_d[:])
            lnw_sb = pp.tile([P, CTX], f32)
            nc.sync.dma_start(lnw_sb, lnw_d[:])
            lnb_sb = pp.tile([P, CTX], f32)
            nc.sync.dma_start(lnb_sb, lnb_d[:])
            sel_sb = pp.tile([P, GT], f32)
            nc.sync.dma_start(sel_sb, sel_d[:])
            selt_sb = pp.tile([P, P], f32)
            nc.sync.dma_start(selt_sb[:GT, :], selt_d[:])
            ident_sb = pp.tile([P, P], f32)
            nc.sync.dma_start(ident_sb, id_d[:])
            ones_sb = pp.tile([P, P], f32r)
            onesf32_sb = pp.tile([P, P], f32)
            nc.vector.memset(onesf32_sb, 1.0)
            nc.vector.tensor_copy(ones_sb, onesf32_sb)
            onef_sb = pp.tile([P, 1], f32)
            nc.vector.memset(onef_sb, 1.0)
            eps_sb = pp.tile([P, 1], f32)
            nc.vector.memset(eps_sb, EPS)

            def psum(name):
                return ps.tile([P, 512], f32, tag="ps", name=name)

            for rep in range(reps):
              for b in range(BPC):
                # ==== phase A: GroupNorm statistics =======================
                stats_all = bp.tile([P, CT, NIT, 6], f32, name=f"stats{b}")
                for ch in range(NIT):
                    xst = xp.tile([P, CT, ITW], f32r, tag="xt", bufs=2,
                                  name=f"xst{b}_{ch}")
                    nc.sync.dma_start(
                        xst, x_d[b, :, :, ch * ITW:(ch + 1) * ITW]
                        .rearrange("t p s -> p t s"))
                    for t in range(CT):
                        nc.vector.bn_stats(out=stats_all[:, t, ch, :],
                                           in_=xst[:, t, :].bitcast(f32))
                mv = bp.tile([P, CT, 2], f32, name=f"mv{b}")
                for t in range(CT):
                    nc.vector.bn_aggr(out=mv[:, t, :], in_=stats_all[:, t])
                # per-channel (mean, E[x^2]) for the group reduce
                statsc = bp.tile([P, CT, 2], f32, name=f"statsc{b}")
                nc.any.tensor_copy(statsc[:, :, 0], mv[:, :, 0])
                nc.vector.tensor_tensor(statsc[:, :, 1], mv[:, :, 0],
                                        mv[:, :, 0], OP.mult)
                nc.vector.tensor_tensor(statsc[:, :, 1], statsc[:, :, 1],
                                        mv[:, :, 1], OP.add)
                # cross-partition group reduce: out8[j, t*2+m] over 16 chans
                ps8 = psum(f"ps8_{b}")
                nc.tensor.matmul(ps8[:GT, :CT * 2], sel_sb,
                                 statsc.rearrange("p a b -> p (a b)"),
                                 start=True, stop=True)
                gst = bp.tile([P, CT, 2], f32, name=f"gst{b}")
                nc.vector.tensor_scalar(gst[:GT].rearrange("j a b -> j (a b)"),
                                        ps8[:GT, :CT * 2], 1.0 / GS, None,
                                        OP.mult)
                g2 = bp.tile([P, CT], f32, name=f"g2_{b}")
                nc.vector.tensor_tensor(g2[:GT], gst[:GT, :, 0], gst[:GT, :, 0],
                                        OP.mult)
                nc.vector.tensor_tensor(g2[:GT], gst[:GT, :, 1], g2[:GT],
                                        OP.subtract)
                nc.scalar.activation(g2[:GT], g2[:GT], AF.Sqrt,
                                     bias=eps_sb[:GT], scale=1.0)
                nc.vector.reciprocal(g2[:GT], g2[:GT])
                bc_in = bp.tile([P, CT, 2], f32, name=f"bc_in{b}")
                nc.any.tensor_copy(bc_in[:GT, :, 0], gst[:GT, :, 0])
                nc.any.tensor_copy(bc_in[:GT, :, 1], g2[:GT])
                # broadcast group stats back to all 128 channel partitions
                psb = psum(f"psb_{b}")
                nc.tensor.matmul(psb[:, :CT * 2], selt_sb[:GT, :],
                                 bc_in[:GT].rearrange("j a b -> j (a b)"),
                                 start=True, stop=True)
                mb = bp.tile([P, CT, 2], f32, name=f"mb{b}")
                nc.any.tensor_copy(mb.rearrange("p a b -> p (a b)"),
                                   psb[:, :CT * 2])
                # A = rstd*gn_w ; Bc = gn_b - mean*A
                ga = bp.tile([P, CT], f32, name=f"ga{b}")
                nc.vector.tensor_tensor(ga, mb[:, :, 1], gnw_sb, OP.mult)
                gb = bp.tile([P, CT], f32, name=f"gb{b}")
                nc.vector.tensor_tensor(gb, mb[:, :, 0], ga, OP.mult)
                nc.vector.tensor_tensor(gb, gnb_sb, gb, OP.subtract)
                # wq' = wqt * A (per c_in channel), in fp32r for the PE
                wqt_b = bp.tile([P, CT, C], f32r, bufs=1, name=f"wqtb{b}")
                for k in range(CT):
                    nc.vector.tensor_scalar(wqt_b[:, k, :], wqt_sb[:, k, :],
                                            ga[:, k:k + 1], None, OP.mult)
                # q0 = wq @ Bc  (fp32, tiny)
                psq0 = psum(f"psq0_{b}")
                for k in range(CT):
                    nc.tensor.matmul(psq0[:1, :C], gb[:, k:k + 1],
                                     wqt_sb[:, k, :], start=(k == 0),
                                     stop=(k == CT - 1))
                q0sb = bp.tile([P, C], f32, name=f"q0sb{b}")
                nc.any.tensor_copy(q0sb[:1, :], psq0[:1, :C])
                # transpose q0 [1, 512] -> [128, 4] via K=1 matmuls
                psq0t = psum(f"psq0t_{b}")
                for m in range(CT):
                    nc.tensor.matmul(psq0t[:, m:m + 1],
                                     q0sb[:1, m * P:(m + 1) * P],
                                     onef_sb[:1, :], start=True, stop=True)
                bqq0 = bp.tile([P, CT], f32, name=f"bqq0{b}")
                nc.vector.tensor_tensor(bqq0, psq0t[:, :CT], bq_sb, OP.add)

                # ==== phase B: LayerNorm(context) + K/V ===================
                ct_sb = bp.tile([P, ST, CTX], f32, bufs=1, tag="ct", name=f"ct{b}")
                nc.sync.dma_start(ct_sb, ctx_d[b].rearrange("t p s -> p t s"))
                stats_ln = bp.tile([P, ST, 3, 6], f32, name=f"statsln{b}")
                for st in range(ST):
                    for c3 in range(3):
                        nc.vector.bn_stats(
                            out=stats_ln[:, st, c3, :],
                            in_=ct_sb[:, st, c3 * 256:(c3 + 1) * 256])
                mv_ln = bp.tile([P, ST, 2], f32, name=f"mvln{b}")
                rs_ln = bp.tile([P, ST], f32, name=f"rsln{b}")
                for st in range(ST):
                    nc.vector.bn_aggr(out=mv_ln[:, st, :], in_=stats_ln[:, st])
                    nc.scalar.activation(rs_ln[:, st:st + 1], mv_ln[:, st, 1:2],
                                         AF.Sqrt, bias=eps_sb, scale=1.0)
                    nc.vector.reciprocal(rs_ln[:, st:st + 1],
                                         rs_ln[:, st:st + 1])
                    # cn = (ct - mean) * rstd, then *ln_w + ln_b (in place)
                    nc.vector.tensor_scalar(ct_sb[:, st, :], ct_sb[:, st, :],
                                            mv_ln[:, st, 0:1],
                                            rs_ln[:, st:st + 1],
                                            OP.subtract, OP.mult)
                    nc.vector.tensor_tensor(ct_sb[:, st, :], ct_sb[:, st, :],
                                            lnw_sb, OP.mult)
                    nc.vector.tensor_tensor(ct_sb[:, st, :], ct_sb[:, st, :],
                                            lnb_sb, OP.add)
                # transpose cn -> cnt [ctx, s]
                cnt = bp.tile([P, KTC, S], f32r, bufs=1, tag="cnt", name=f"cnt{b}")
                for st in range(ST):
                    for kc in range(KTC):
                        pst = psum(f"pst{b}_{st}_{kc}")
                        nc.tensor.transpose(pst[:, :P],
                                            ct_sb[:, st, kc * P:(kc + 1) * P],
                                            ident_sb)
                        nc.any.tensor_copy(cnt[:, kc, st * P:(st + 1) * P],
                                           pst[:, :P])
                # K^T [c, j] with bias bk
                kt_sb = bp.tile([P, CT, S], f32r, name=f"kt{b}")
                for m in range(CT):
                    psk = psum(f"psk{b}_{m}")
                    for k in range(KTC):
                        nc.tensor.matmul(psk[:, :S],
                                         wkvt_sb[:, k, m * P:(m + 1) * P],
                                         cnt[:, k, :], start=(k == 0),
                                         stop=(k == KTC - 1))
                    nc.scalar.activation(kt_sb[:, m, :], psk[:, :S],
                                         AF.Identity, bias=bk_sb[:, m:m + 1],
                                         scale=1.0)
                # V [j, c] with bias bv
                v_sb = bp.tile([P, ST, C], f32r, name=f"v{b}")
                for jm in range(ST):
                    psv = psum(f"psv{b}_{jm}")
                    for k in range(KTC):
                        nc.tensor.matmul(psv[:, :C],
                                         cnt[:, k, jm * P:(jm + 1) * P],
                                         wkvt_sb[:, k, C:2 * C],
                                         start=(k == 0), stop=(k == KTC - 1))
                    nc.vector.tensor_tensor(v_sb[:, jm, :], psv[:, :C],
                                            bv_sb, OP.add)

                # ==== phase C: attention, streamed over hw i-tiles ========
                for it in range(NIT):
                    i0 = it * ITW
                    xti = xp.tile([P, CT, ITW], f32r, tag="xt", bufs=2,
                                  name=f"xti{b}_{it}")
                    nc.sync.dma_start(
                        xti, x_d[b, :, :, i0:i0 + ITW]
                        .rearrange("t p s -> p t s"))
                    # Q = wq' @ x + (bq + q0)
                    q_sb = ip.tile([P, CT, ITW], f32r, tag="q", bufs=2,
                                   name=f"q{b}_{it}")
                    for m in range(CT):
                        psq = psum(f"psq{b}_{it}_{m}")
                        for k in range(CT):
                            nc.tensor.matmul(psq, wqt_b[:, k, m * P:(m + 1) * P],
                                             xti[:, k, :], start=(k == 0),
                                             stop=(k == CT - 1))
                        nc.scalar.activation(q_sb[:, m, :], psq, AF.Identity,
                                             bias=bqq0[:, m:m + 1], scale=1.0)
                    # sim^T then exp
                    expt = ip.tile([P, ST, ITW], f32r, tag="expt", bufs=2,
                                   name=f"expt{b}_{it}")
                    for jm in range(ST):
                        pss = psum(f"pss{b}_{it}_{jm}")
                        for k in range(CT):
                            nc.tensor.matmul(pss, kt_sb[:, k, jm * P:(jm + 1) * P],
                                             q_sb[:, k, :], start=(k == 0),
                                             stop=(k == CT - 1))
                        nc.scalar.activation(expt[:, jm, :], pss, AF.Exp,
                                             scale=SCALE)
                    # denominator, replicated across partitions via ones-matmul
                    psd = psum(f"psd{b}_{it}")
                    for jm in range(ST):
                        nc.tensor.matmul(psd, ones_sb,
                                         expt[:, jm, :], start=(jm == 0),
                                         stop=(jm == ST - 1))
                    recip = ip.tile([P, ITW], f32r, tag="recip", bufs=2,
                                    name=f"recip{b}_{it}")
                    with nc.allow_low_precision(
                            reason="softmax denom rounded to f32r for PE"):
                        nc.vector.reciprocal(recip, psd)
                    for jm in range(ST):
                        nc.vector.tensor_tensor(expt[:, jm, :], expt[:, jm, :],
                                                recip, OP.mult)
                    # out^T = V^T @ attn^T
                    ot_sb = ip.tile([P, CT, ITW], f32r, tag="expt", bufs=2,
                                    name=f"ot{b}_{it}")
                    for m in range(CT):
                        pso = psum(f"pso{b}_{it}_{m}")
                        for jm in range(ST):
                            nc.tensor.matmul(pso, v_sb[:, jm, m * P:(m + 1) * P],
                                             expt[:, jm, :], start=(jm == 0),
                                             stop=(jm == ST - 1))
                        nc.any.tensor_copy(ot_sb[:, m, :], pso)
                    # final projection + bias + residual
                    fout = ip.tile([P, CT, ITW], f32, tag="q", bufs=2,
                                   name=f"fout{b}_{it}")
                    for m in range(CT):
                        psf = psum(f"psf{b}_{it}_{m}")
                        for k in range(CT):
                            nc.tensor.matmul(psf, wot_sb[:, k, m * P:(m + 1) * P],
                                             ot_sb[:, k, :], start=(k == 0),
                                             stop=(k == CT - 1))
                        nc.any.tensor_scalar(fout[:, m, :], psf,
                                             bo_sb[:, m:m + 1], None, OP.add)
                        nc.vector.tensor_tensor(fout[:, m, :], fout[:, m, :],
                                                xti[:, m, :].bitcast(f32),
                                                OP.add)
                    nc.sync.dma_start(
                        out_d[b, :, :, i0:i0 + ITW].rearrange("t p s -> p t s"),
                        fout)

    nc.finalize()
    _CACHE[key] = nc
    return nc


def make_in_maps(inputs):
    """Host-side preprocessing: shard + relayout inputs for the 8 cores."""
    f32 = np.float32
    x = np.ascontiguousarray(inputs["x"], dtype=f32)
    context = np.ascontiguousarray(inputs["context"], dtype=f32)
    wq = np.asarray(inputs["wq"], dtype=f32)
    wkv = np.asarray(inputs["wkv"], dtype=f32)
    wo = np.asarray(inputs["wo"], dtype=f32)

    def chan_part(v):
        return np.ascontiguousarray(np.asarray(v, f32).reshape(CT, P).T)

    GT = G // CT
    sel = np.zeros((P, GT), f32)
    for p in range(P):
        sel[p, p // GS] = 1.0
    shared = {
        "wqt": np.ascontiguousarray(wq.T).reshape(CT, P, C),
        "wkvt": np.ascontiguousarray(wkv.T).reshape(KTC, P, 2 * C),
        "wot": np.ascontiguousarray(wo.T).reshape(CT, P, C),
        "bq_p": chan_part(inputs["bq"]),
        "bo_p": chan_part(inputs["bo"]),
        "bk_p": chan_part(np.asarray(inputs["bkv"], f32)[:C]),
        "gnw_p": chan_part(inputs["gn_w"]),
        "gnb_p": chan_part(inputs["gn_b"]),
        "bv_b": np.ascontiguousarray(
            np.broadcast_to(np.asarray(inputs["bkv"], f32)[C:], (P, C))),
        "lnw_b": np.ascontiguousarray(
            np.broadcast_to(np.asarray(inputs["ln_w"], f32), (P, CTX))),
        "lnb_b": np.ascontiguousarray(
            np.broadcast_to(np.asarray(inputs["ln_b"], f32), (P, CTX))),
        "sel": sel,
        "selt": np.ascontiguousarray(sel.T),
        "ident": np.eye(P, dtype=f32),
    }
    xs = x.reshape(NCORES, BPC, CT, P, HW)
    cs = context.reshape(NCORES, BPC, ST, P, CTX)
    return [dict(shared, x=np.ascontiguousarray(xs[c]),
                 ctx=np.ascontiguousarray(cs[c])) for c in range(NCORES)]


def kernel(**inputs) -> np.ndarray:
    from concourse.bass_utils import run_bass_kernel_spmd

    nc = build_nc()
    in_maps = make_in_maps(inputs)
    res = run_bass_kernel_spmd(nc, in_maps, list(range(NCORES)))
    outs = [res.results[c]["out"] for c in range(NCORES)]
    full = np.stack(outs, axis=0).reshape(B, C, HGT, WID)
    return full.astype(np.float32)

